# revision 22
# baseline (speedup 1.0000x reference)
"""BitLinear (fake-quant straight-through) Trainium2 kernel.

Math (per the reference nn module):
  dqx = round(x * s_x) / s_x         s_x = 127 / clip(rowabsmax(x), 1e-5)   (per token row)
  dqw = clip(round(w * s_w), -1, 1) / s_w    s_w = 1 / clip(mean(|w|), 1e-5)  (per tensor)
  out = dqx @ dqw.T + bias

Key facts this kernel exploits:
  * round(x*s_x) is an integer in [-127, 127] and clip(round(w*s_w)) is in
    {-1, 0, 1}; both are EXACT in bf16, and the matmul accumulates in fp32
    PSUM where all partial sums (<= 2^17) are exact integers.  So the heavy
    matmul runs at bf16 PE rate with zero quantization-path error; the
    per-token / per-tensor scales are applied to the (exact) integer matmul
    result afterwards.
  * round-half-even == fp32 RNE, so `round(v)` is computed exactly as
    `(v + 1.5*2^23) - 1.5*2^23` with two fp32 ALU stages (no Round op needed).
  * the bias lands in PSUM pre-scale as three extra contraction rows
    ([e_hi, e_lo, e_hi] x [b_hi, b_hi, b_lo], e = per-token 1/out-scale split
    bf16 hi/lo) so no engine downstream of the matmul does elementwise work
    beyond the single scaled PSUM evacuation.  (A gpsimd bias-add variant
    measured SLOWER: it saturates the in-order gpsimd queue and triples HAM
    power-throttle windows.)

Sharding: data parallel over the batch dim; core i computes batch element i
with the full weight.  No collectives; the host scatters x and gathers out.

Pipeline structure: tokens are processed in "quads" (4 x 128 = 512 tokens).
Engine assignment (each pipeline stage owns an engine; upstream x-prep never
shares an engine with anything downstream of the matmul):
  scalar : x input DMA triggers (own HWDGE ring), one-time weight scale,
           PSUM evacuation with per-token output scale
  vector : absmax reduce, scales, quantize (round via magic), weight round/clip
  sync   : xbar transposes only (x quads and the one-time weight tiles)
  tensor : matmuls (bf16 exact-integer)
  gpsimd : one-time weight-chunk input DMA, per-s output stores (SWDGE)

Startup is latency-optimized: x quad 0 and the weight tensor stream in on
separate DMA rings; quad 0's x prep is split in two halves; its matmuls run
as h0-then-h1 phases (each kt-loop interleaves a PAIR of PSUM banks --
back-to-back matmuls into the SAME bank run at half rate) so they can begin
as soon as the first four weight n-tiles are transposed.  Weight chunks 0-1
quantize on DVE (fast path for phase h0), chunks 2-3 on the otherwise-idle
gpsimd so DVE can move straight on to quad-1 prep.

The PE's power controller (HAM) is a duty-cycle ratchet: it grants full
clock only after sustained matmul activity and regresses to ~50% duty after
idle periods (measured: 6.8us full / 10.2us half alternation growing to
80us+ full windows).  A stream of small dummy matmuls on zeros pre-warms the
ratchet during the load/prep phase so the real matmuls start at full rate,
with a second short filler batch bridging the phase-A -> phase-B weight wait.

The per-tensor weight scale s_w is the one input-derived scalar computed on
the host (it must match the reference's fp32 mean reduction to ~1 ulp, which
an on-device sequential reduction cannot guarantee; a 1e-6 relative error in
s_w flips ternary weights and produces visible output error).  It is passed
in through a small constants tensor, so the compiled program is input-
independent.
"""

import numpy as np

from concourse import bacc, bass, mybir, tile
from concourse.bass_utils import run_bass_kernel_spmd

F32 = mybir.dt.float32
BF16 = mybir.dt.bfloat16
ALU = mybir.AluOpType
ACTF = mybir.ActivationFunctionType

MAGIC = 12582912.0  # 1.5 * 2**23: fp32 RNE round-to-integer constant
EPS = 1e-05

B, S, K, N = 8, 4096, 1024, 1024
N_CORES = 8
QS = 4  # token tiles per quad


def build(s_tokens=S, k=K, n=N):
    """Build the single-core SPMD program: x[s_tokens,k] @ w[n,k]^T quantized."""
    nc = bacc.Bacc("TRN2", target_bir_lowering=False, debug=False)

    x_d = nc.dram_tensor("x", [s_tokens, k], F32, kind="ExternalInput").ap()
    w_d = nc.dram_tensor("w", [n, k], F32, kind="ExternalInput").ap()
    brows_d = nc.dram_tensor("brows", [3, n], BF16, kind="ExternalInput").ap()
    consts_d = nc.dram_tensor("consts", [128, 8], F32, kind="ExternalInput").ap()
    out_d = nc.dram_tensor("out", [s_tokens, n], F32, kind="ExternalOutput").ap()

    KT = k // 128          # contraction tiles
    NT = n // 128          # weight row tiles
    NH = n // 512          # psum-bank halves of the output feature dim
    NQ = s_tokens // (128 * QS)  # quads
    WC = 4                 # weight chunks (2 n-tiles each)
    KE = k + 128           # one extra transpose chunk for the bias e-rows

    x_q = x_d.rearrange("(q s p) k -> q p s k", s=QS, p=128)
    out_q = out_d.rearrange("(q s p) n -> q p s n", s=QS, p=128)
    w_c = w_d.rearrange("(c t p) k -> c p t k", t=2, p=128)

    with tile.TileContext(nc) as tc:
        with (
            tc.tile_pool(name="static", bufs=1) as static,
            tc.tile_pool(name="wpool", bufs=2) as wpool,
            tc.tile_pool(name="xpool", bufs=3) as xpool,
            tc.tile_pool(name="qpool", bufs=2) as qpool,
            tc.tile_pool(name="qtpool", bufs=2) as qtpool,
            tc.tile_pool(name="opool", bufs=6) as opool,
            tc.tile_pool(name="vpool", bufs=8) as vpool,
            tc.tile_pool(name="psum", bufs=4, space="PSUM") as psum_pool,
        ):
            consts = static.tile([128, 8], F32)
            nc.scalar.dma_start(consts[:], consts_d[:])
            # bias rows for the bias-via-matmul contraction: [b_hi, b_lo, b_hi]
            brows = static.tile([3, n], BF16)
            nc.scalar.dma_start(brows[:], brows_d[:])

            # qwT[kpart, nt, kt, n128] = quantized weight, transposed:
            # k on partitions; [nt, kt] outer so each n-tile's transpose
            # writes a contiguous block
            qwT = static.tile([128, NT, KT, 128], BF16)

            sw_ap = consts[:, 0:1]    # s_w  (weight quant scale)
            k1 = consts[:, 2:3]       # (1/s_w) / 127  (output scale factor)
            k2 = consts[:, 3:4]       # 127 * s_w  (= 1/k1: bias row scale)
            neg_magic = consts[:, 4:5]  # -MAGIC (bias operand for ACT Sign)

            # ---- input loads: x quad 0 (two halves, for startup latency)
            # and quad 1 on the scalar HWDGE ring; weight chunks on the
            # gpsimd SWDGE ring so the two streams proceed in parallel.
            x_tiles = {}
            x0 = xpool.tile([128, QS, k], F32, name="x_s", tag="x_s")
            x_tiles[0] = x0
            nc.gpsimd.dma_start(x0[:, 0:2, :], x_q[0][:, 0:2, :])
            nc.scalar.dma_start(x0[:, 2:4, :], x_q[0][:, 2:4, :])

            w_tiles = []
            for c in range(WC):
                w_s = wpool.tile([128, 2, k], F32, name="w_s", tag="w_s", bufs=4)
                nc.gpsimd.dma_start(w_s[:], w_c[c])
                w_tiles.append(w_s)

            # x1 and the look-ahead loads ride the gpsimd (store) ring: a
            # load trigger that waits on a tile buffer must never sit in
            # front of the PSUM-evacuation stream on the scalar ring.
            x1 = xpool.tile([128, QS, k], F32, name="x_s", tag="x_s")
            x_tiles[1] = x1
            nc.gpsimd.dma_start(x1[:], x_q[1])

            # ---- x prep (DVE only); the absmax reduce is split in two so
            # the first half can start as soon as the first of the quad's
            # two load DMAs lands.
            def prep_half(x_s, qx, lo, hi, split_reduce=False):
                m = hi - lo
                c_ = vpool.tile([128, m], F32, name="c")
                if split_reduce:
                    h2 = (lo + hi) // 2
                    nc.vector.tensor_reduce(
                        c_[:, 0:h2 - lo], x_s[:, lo:h2, :], mybir.AxisListType.X,
                        ALU.max, apply_absolute_value=True,
                    )
                    nc.vector.tensor_reduce(
                        c_[:, h2 - lo:m], x_s[:, h2:hi, :], mybir.AxisListType.X,
                        ALU.max, apply_absolute_value=True,
                    )
                else:
                    nc.vector.tensor_reduce(
                        c_[:], x_s[:, lo:hi, :], mybir.AxisListType.X, ALU.max,
                        apply_absolute_value=True,
                    )
                cc = vpool.tile([128, m], F32, name="cc")
                nc.vector.tensor_scalar_max(cc[:], c_[:], EPS)
                rc = vpool.tile([128, m], F32, name="rc")
                nc.vector.reciprocal(rc[:], cc[:])
                ss = vpool.tile([128, m], F32, name="ss")
                nc.vector.tensor_scalar_mul(ss[:], rc[:], 127.0)
                fs = vpool.tile([128, m], F32, name="fs", tag="fs", bufs=4)
                nc.vector.tensor_scalar_mul(fs[:], cc[:], k1)
                # e = 1/fs = rc*k2 split into bf16 hi+lo, fed into the matmul
                # as extra contraction rows so the bias lands in PSUM pre-scale
                ef = vpool.tile([128, m], F32, name="ef")
                nc.vector.tensor_scalar_mul(ef[:], rc[:], k2)
                ehi = vpool.tile([128, m], BF16, name="ehi")
                nc.vector.tensor_copy(ehi[:], ef[:])
                elo = vpool.tile([128, m], BF16, name="elo")
                nc.vector.tensor_tensor(elo[:], ef[:], ehi[:], ALU.subtract)
                for s in range(lo, hi):
                    nc.vector.tensor_scalar(
                        x_s[:, s, :], x_s[:, s, :], ss[:, s - lo:s - lo + 1],
                        MAGIC, ALU.mult, ALU.add,
                    )
                # round(x*s_x) via magic constant, to bf16 into cols 0..k;
                # cols k..k+3 get the bias e-rows so the quad transpose also
                # produces the bias lhsT.
                nc.vector.tensor_scalar_sub(
                    qx[:, lo:hi, 0:k], x_s[:, lo:hi, :], MAGIC
                )
                nc.vector.tensor_copy(qx[:, lo:hi, k:k + 1], ehi[:])
                nc.vector.tensor_copy(qx[:, lo:hi, k + 1:k + 2], ehi[:])
                nc.vector.tensor_copy(qx[:, lo:hi, k + 2:k + 3], elo[:])
                # the rest of the extra chunk is never read by matmuls, but
                # the transpose streams it; keep it initialized
                nc.vector.memset(qx[:, lo:hi, k + 3:KE], 0.0)
                return fs

            qx0 = qpool.tile([128, QS, KE], BF16, name="qx", tag="qx")
            fs0 = prep_half(x0, qx0, 0, QS, split_reduce=True)

            # transpose: [128s, QS*KE] -> [128k, QS*(KT+1), 128s],
            # chunk j = s*(KT+1)+kt
            qxT0 = qtpool.tile([128, QS, KT + 1, 128], BF16, name="qxT", tag="qxT")
            nc.sync.dma_start_transpose(qxT0[:], qx0[:])

            # ---- weight quantization + transpose (one-time) ----
            # Three exact ACT passes per n-tile (DVE stays free for x prep):
            #   tw  = fl(w*s_w)              (Copy, scale)
            #   tw  = fl(tw + MAGIC)         (Copy, bias)   -> RNE round-to-int + M
            #   qw  = Sign(tw - MAGIC)       (Sign, bias)
            # tw - MAGIC is an exact small integer, and clip(i,-1,1) == Sign(i)
            # for integers, so this matches the reference's round-then-clip
            # with the same rounding points.  Per-n-tile granularity keeps the
            # chain latency low under the tile scheduler's breadth-first
            # ordering.
            for c in range(WC):
                w_s = w_tiles[c]
                tw = wpool.tile([128, 2, k], F32, name="tw", tag="tw")
                qw = wpool.tile([128, 2, k], BF16, name="qw", tag="qw")
                for t in range(2):
                    nc.scalar.activation(tw[:, t], w_s[:, t], ACTF.Copy, scale=sw_ap)
                    nc.scalar.activation(tw[:, t], tw[:, t], ACTF.Copy, bias=MAGIC)
                    nc.scalar.activation(qw[:, t], tw[:, t], ACTF.Sign, bias=neg_magic)
                    # one batched xbar transpose per n-tile: [128n, 1024k] ->
                    # [128k, KT, 128n] (out row r = j*128+p, verified on hw)
                    nc.sync.dma_start_transpose(qwT[:, 2 * c + t], qw[:, t, :])

            # ---- main pipeline over token quads ----
            def emit_bias_mm(ps_list, qxT, s):
                for h in range(NH):
                    nc.tensor.matmul(
                        ps_list[h][:],
                        qxT[0:3, s, KT, :],
                        brows[:, h * 512:(h + 1) * 512],
                        start=False,
                        stop=True,
                    )

            def emit_output(q, s, ps_list, fs_ap):
                o = opool.tile([128, n], F32, name="o", tag="o")
                for h in range(NH):
                    nc.scalar.activation(
                        o[:, h * 512:(h + 1) * 512], ps_list[h][:],
                        ACTF.Copy, scale=fs_ap,
                    )
                nc.gpsimd.dma_start(out_q[q][:, s, :], o[:])

            for q in range(NQ):
                # look-ahead x load (2 quads ahead; xpool bufs=3)
                if q + 2 < NQ:
                    xn = xpool.tile([128, QS, k], F32, name="x_s", tag="x_s")
                    x_tiles[q + 2] = xn
                    nc.gpsimd.dma_start(xn[:], x_q[q + 2])

                if q == 0:
                    # phase-split matmuls: all h=0 first (needs only the
                    # first 4 transposed weight tiles), then all h=1.  Each
                    # kt loop interleaves a PAIR of psum banks so no two
                    # consecutive matmuls hit the same bank.
                    ps = {}
                    for h in range(NH):
                        for s0 in (0, 2):
                            for s in (s0, s0 + 1):
                                ps[(s, h)] = psum_pool.tile(
                                    [128, 512], F32, name=f"ps{h}", tag=f"ps{h}"
                                )
                            for kt in range(KT):
                                for s in (s0, s0 + 1):
                                    nc.tensor.matmul(
                                        ps[(s, h)][:],
                                        qxT0[:, s, kt, :],
                                        qwT[:, 4 * h:4 * h + 4, kt, :],
                                        start=(kt == 0),
                                        stop=False,
                                    )
                        # this h's bias rows right after its phase: real PE
                        # work that bridges the wait for the other phase's
                        # weight transposes
                        for s in range(QS):
                            nc.tensor.matmul(
                                ps[(s, h)][:],
                                qxT0[0:3, s, KT, :],
                                brows[:, h * 512:(h + 1) * 512],
                                start=False,
                                stop=True,
                            )
                    for s in range(QS):
                        ps_list = [ps[(s, h)] for h in range(NH)]
                        emit_output(q, s, ps_list, fs0[:, s:s + 1])
                    continue

                # steady-state quad: batched prep on DVE, one transpose,
                # kt-interleaved matmuls per s-group.
                x_s = x_tiles[q]
                qx = qpool.tile([128, QS, KE], BF16, name="qx", tag="qx")
                fs = prep_half(x_s, qx, 0, QS)
                qxT = qtpool.tile([128, QS, KT + 1, 128], BF16, name="qxT", tag="qxT")
                nc.sync.dma_start_transpose(qxT[:], qx[:])

                for s in range(QS):
                    ps_list = [
                        psum_pool.tile([128, 512], F32, name=f"ps{h}", tag=f"ps{h}")
                        for h in range(NH)
                    ]
                    for kt in range(KT):
                        for h in range(NH):
                            nc.tensor.matmul(
                                ps_list[h][:],
                                qxT[:, s, kt, :],
                                qwT[:, 4 * h:4 * h + 4, kt, :],
                                start=(kt == 0),
                                stop=False,
                            )
                    emit_bias_mm(ps_list, qxT, s)
                    emit_output(q, s, ps_list, fs[:, s:s + 1])

    nc.compile()
    return nc


def host_consts(weight):
    """The one input-derived scalar: s_w, matching the reference's fp32 mean."""
    try:
        import jax
        import jax.numpy as jnp

        with jax.default_device(jax.devices("cpu")[0]):
            mean_abs = np.float32(
                jax.device_get(jnp.mean(jnp.abs(jnp.asarray(weight, dtype=jnp.float32))))
            )
    except Exception:
        mean_abs = np.float32(np.mean(np.abs(weight), dtype=np.float32))
    mean_c = np.maximum(mean_abs, np.float32(EPS))
    sw = np.float32(1.0) / mean_c          # s_w, the weight quant scale
    wdiv = np.float32(1.0) / sw            # 1/s_w (the ternary unit value)
    k1 = wdiv / np.float32(127.0)          # output scale = cc * k1
    k2 = np.float32(1.0) / k1              # bias e-row scale (= 1/k1)
    row = np.zeros((8,), np.float32)
    row[0], row[1], row[2], row[3] = sw, np.float32(127.0), k1, k2
    row[4] = np.float32(-MAGIC)
    return np.tile(row[None, :], (128, 1)).copy()


def host_brows(bias):
    """bf16 hi/lo split of the bias for the bias-via-matmul rows."""
    import ml_dtypes

    b = np.asarray(bias, dtype=np.float32)
    b_hi = b.astype(ml_dtypes.bfloat16)
    b_lo = (b - b_hi.astype(np.float32)).astype(ml_dtypes.bfloat16)
    return np.stack([b_hi, b_lo, b_hi], axis=0)


_NC_CACHE = {}


def _get_nc():
    if "nc" not in _NC_CACHE:
        _NC_CACHE["nc"] = build()
    return _NC_CACHE["nc"]


def make_in_maps(x, weight, bias):
    x = np.ascontiguousarray(x, dtype=np.float32)
    weight = np.ascontiguousarray(weight, dtype=np.float32)
    bias = np.ascontiguousarray(bias, dtype=np.float32)
    consts = host_consts(weight)
    brows = host_brows(bias)
    return [
        {"x": x[i], "w": weight, "brows": brows, "consts": consts}
        for i in range(N_CORES)
    ]


def kernel(x, weight, bias, **kwargs):
    nc = _get_nc()
    in_maps = make_in_maps(x, weight, bias)
    last_err = None
    for _attempt in range(3):
        try:
            res = run_bass_kernel_spmd(nc, in_maps, list(range(N_CORES)))
            return np.stack([res.results[i]["out"] for i in range(N_CORES)], axis=0)
        except Exception as e:  # transient NRT device errors: retry
            last_err = e
    raise last_err


# revision 27
# speedup vs baseline: 1.0281x; 1.0281x over previous
"""BitLinear (fake-quant straight-through) Trainium2 kernel.

Math (per the reference nn module):
  dqx = round(x * s_x) / s_x         s_x = 127 / clip(rowabsmax(x), 1e-5)   (per token row)
  dqw = clip(round(w * s_w), -1, 1) / s_w    s_w = 1 / clip(mean(|w|), 1e-5)  (per tensor)
  out = dqx @ dqw.T + bias

Key facts this kernel exploits:
  * round(x*s_x) is an integer in [-127, 127] and clip(round(w*s_w)) is in
    {-1, 0, 1}; both are EXACT in bf16, and the matmul accumulates in fp32
    PSUM where all partial sums (<= 2^17) are exact integers.  So the heavy
    matmul runs at bf16 PE rate with zero quantization-path error; the
    per-token / per-tensor scales are applied to the (exact) integer matmul
    result afterwards.
  * round-half-even == fp32 RNE, so `round(v)` is computed exactly as
    `(v + 1.5*2^23) - 1.5*2^23` with two fp32 ALU stages (no Round op needed).
  * the bias lands in PSUM pre-scale as three extra contraction rows
    ([e_hi, e_lo, e_hi] x [b_hi, b_hi, b_lo], e = per-token 1/out-scale split
    bf16 hi/lo) so no engine downstream of the matmul does elementwise work
    beyond the single scaled PSUM evacuation.  (A gpsimd bias-add variant
    measured SLOWER: it saturates the in-order gpsimd queue and triples HAM
    power-throttle windows.)

Sharding: data parallel over the batch dim; core i computes batch element i
with the full weight.  No collectives; the host scatters x and gathers out.

Pipeline structure: tokens are processed in "quads" (4 x 128 = 512 tokens).
Engine assignment (each pipeline stage owns an engine; upstream x-prep never
shares an engine with anything downstream of the matmul):
  scalar : x input DMA triggers (own HWDGE ring), one-time weight scale,
           PSUM evacuation with per-token output scale
  vector : absmax reduce, scales, quantize (round via magic), weight round/clip
  sync   : xbar transposes only (x quads and the one-time weight tiles)
  tensor : matmuls (bf16 exact-integer)
  gpsimd : one-time weight-chunk input DMA, per-s output stores (SWDGE)

Startup is latency-optimized: x quad 0 and the weight tensor stream in on
separate DMA rings; quad 0's x prep is split in two halves; its matmuls run
as h0-then-h1 phases (each kt-loop interleaves a PAIR of PSUM banks --
back-to-back matmuls into the SAME bank run at half rate) so they can begin
as soon as the first four weight n-tiles are transposed.  Weight chunks 0-1
quantize on DVE (fast path for phase h0), chunks 2-3 on the otherwise-idle
gpsimd so DVE can move straight on to quad-1 prep.

The PE's power controller (HAM) is a duty-cycle ratchet: it grants full
clock only after sustained matmul activity and regresses to ~50% duty after
idle periods (measured: 6.8us full / 10.2us half alternation growing to
80us+ full windows).  A stream of small dummy matmuls on zeros pre-warms the
ratchet during the load/prep phase so the real matmuls start at full rate,
with a second short filler batch bridging the phase-A -> phase-B weight wait.

The per-tensor weight scale s_w is the one input-derived scalar computed on
the host (it must match the reference's fp32 mean reduction to ~1 ulp, which
an on-device sequential reduction cannot guarantee; a 1e-6 relative error in
s_w flips ternary weights and produces visible output error).  It is passed
in through a small constants tensor, so the compiled program is input-
independent.
"""

import numpy as np

from concourse import bacc, bass, mybir, tile
from concourse.bass_utils import run_bass_kernel_spmd

F32 = mybir.dt.float32
BF16 = mybir.dt.bfloat16
ALU = mybir.AluOpType
ACTF = mybir.ActivationFunctionType

MAGIC = 12582912.0  # 1.5 * 2**23: fp32 RNE round-to-integer constant
EPS = 1e-05

B, S, K, N = 8, 4096, 1024, 1024
N_CORES = 8
QS = 4  # token tiles per quad


def build(s_tokens=S, k=K, n=N):
    """Build the single-core SPMD program: x[s_tokens,k] @ w[n,k]^T quantized."""
    nc = bacc.Bacc("TRN2", target_bir_lowering=False, debug=False)

    x_d = nc.dram_tensor("x", [s_tokens, k], F32, kind="ExternalInput").ap()
    w_d = nc.dram_tensor("w", [n, k], F32, kind="ExternalInput").ap()
    brows_d = nc.dram_tensor("brows", [3, n], BF16, kind="ExternalInput").ap()
    consts_d = nc.dram_tensor("consts", [128, 8], F32, kind="ExternalInput").ap()
    out_d = nc.dram_tensor("out", [s_tokens, n], F32, kind="ExternalOutput").ap()

    KT = k // 128          # contraction tiles
    NT = n // 128          # weight row tiles
    NH = n // 512          # psum-bank halves of the output feature dim
    NQ = s_tokens // (128 * QS)  # quads
    WC = 4                 # weight chunks (2 n-tiles each)
    KE = k + 128           # one extra transpose chunk for the bias e-rows

    x_q = x_d.rearrange("(q s p) k -> q p s k", s=QS, p=128)
    out_q = out_d.rearrange("(q s p) n -> q p s n", s=QS, p=128)
    w_c = w_d.rearrange("(c t p) k -> c p t k", t=2, p=128)

    with tile.TileContext(nc) as tc:
        with (
            tc.tile_pool(name="static", bufs=1) as static,
            tc.tile_pool(name="wpool", bufs=2) as wpool,
            tc.tile_pool(name="xpool", bufs=3) as xpool,
            tc.tile_pool(name="qpool", bufs=2) as qpool,
            tc.tile_pool(name="qtpool", bufs=2) as qtpool,
            tc.tile_pool(name="opool", bufs=6) as opool,
            tc.tile_pool(name="vpool", bufs=8) as vpool,
            tc.tile_pool(name="psum", bufs=4, space="PSUM") as psum_pool,
        ):
            consts = static.tile([128, 8], F32)
            nc.scalar.dma_start(consts[:], consts_d[:])
            # bias rows for the bias-via-matmul contraction: [b_hi, b_lo, b_hi]
            brows = static.tile([3, n], BF16)
            nc.scalar.dma_start(brows[:], brows_d[:])

            # qwT[kpart, nt, kt, n128] = quantized weight, transposed:
            # k on partitions; [nt, kt] outer so each n-tile's transpose
            # writes a contiguous block
            qwT = static.tile([128, NT, KT, 128], BF16)

            sw_ap = consts[:, 0:1]    # s_w  (weight quant scale)
            k1 = consts[:, 2:3]       # (1/s_w) / 127  (output scale factor)
            k2 = consts[:, 3:4]       # 127 * s_w  (= 1/k1: bias row scale)
            neg_magic = consts[:, 4:5]  # -MAGIC (bias operand for ACT Sign)

            # ---- input loads: x quad 0 (two halves, for startup latency)
            # and quad 1 on the scalar HWDGE ring; weight chunks on the
            # gpsimd SWDGE ring so the two streams proceed in parallel.
            x_tiles = {}
            x0 = xpool.tile([128, QS, k], F32, name="x_s", tag="x_s")
            x_tiles[0] = x0
            nc.gpsimd.dma_start(x0[:, 0:2, :], x_q[0][:, 0:2, :])
            nc.scalar.dma_start(x0[:, 2:4, :], x_q[0][:, 2:4, :])

            x1 = xpool.tile([128, QS, k], F32, name="x_s", tag="x_s")
            x_tiles[1] = x1
            nc.scalar.dma_start(x1[:], x_q[1])

            w_tiles = []
            for c in range(WC):
                w_s = wpool.tile([128, 2, k], F32, name="w_s", tag="w_s", bufs=4)
                nc.gpsimd.dma_start(w_s[:], w_c[c])
                w_tiles.append(w_s)

            # ---- x prep (DVE only); the absmax reduce is split in two so
            # the first half can start as soon as the first of the quad's
            # two load DMAs lands.
            def prep_half(x_s, qx, lo, hi, split_reduce=False):
                m = hi - lo
                c_ = vpool.tile([128, m], F32, name="c")
                if split_reduce:
                    h2 = (lo + hi) // 2
                    nc.vector.tensor_reduce(
                        c_[:, 0:h2 - lo], x_s[:, lo:h2, :], mybir.AxisListType.X,
                        ALU.max, apply_absolute_value=True,
                    )
                    nc.vector.tensor_reduce(
                        c_[:, h2 - lo:m], x_s[:, h2:hi, :], mybir.AxisListType.X,
                        ALU.max, apply_absolute_value=True,
                    )
                else:
                    nc.vector.tensor_reduce(
                        c_[:], x_s[:, lo:hi, :], mybir.AxisListType.X, ALU.max,
                        apply_absolute_value=True,
                    )
                cc = vpool.tile([128, m], F32, name="cc")
                nc.vector.tensor_scalar_max(cc[:], c_[:], EPS)
                rc = vpool.tile([128, m], F32, name="rc")
                nc.vector.reciprocal(rc[:], cc[:])
                ss = vpool.tile([128, m], F32, name="ss")
                nc.vector.tensor_scalar_mul(ss[:], rc[:], 127.0)
                fs = vpool.tile([128, m], F32, name="fs", tag="fs", bufs=4)
                nc.vector.tensor_scalar_mul(fs[:], cc[:], k1)
                # e = 1/fs = rc*k2 split into bf16 hi+lo, fed into the matmul
                # as extra contraction rows so the bias lands in PSUM pre-scale
                ef = vpool.tile([128, m], F32, name="ef")
                nc.vector.tensor_scalar_mul(ef[:], rc[:], k2)
                ehi = vpool.tile([128, m], BF16, name="ehi")
                nc.vector.tensor_copy(ehi[:], ef[:])
                elo = vpool.tile([128, m], BF16, name="elo")
                nc.vector.tensor_tensor(elo[:], ef[:], ehi[:], ALU.subtract)
                for s in range(lo, hi):
                    nc.vector.tensor_scalar(
                        x_s[:, s, :], x_s[:, s, :], ss[:, s - lo:s - lo + 1],
                        MAGIC, ALU.mult, ALU.add,
                    )
                # round(x*s_x) via magic constant, to bf16 into cols 0..k;
                # cols k..k+3 get the bias e-rows so the quad transpose also
                # produces the bias lhsT.
                nc.vector.tensor_scalar_sub(
                    qx[:, lo:hi, 0:k], x_s[:, lo:hi, :], MAGIC
                )
                nc.vector.tensor_copy(qx[:, lo:hi, k:k + 1], ehi[:])
                nc.vector.tensor_copy(qx[:, lo:hi, k + 1:k + 2], ehi[:])
                nc.vector.tensor_copy(qx[:, lo:hi, k + 2:k + 3], elo[:])
                # the rest of the extra chunk is never read by matmuls, but
                # the transpose streams it; keep it initialized
                nc.vector.memset(qx[:, lo:hi, k + 3:KE], 0.0)
                return fs

            qx0 = qpool.tile([128, QS, KE], BF16, name="qx", tag="qx")
            fs0a = prep_half(x0, qx0, 0, 2)
            fs0b = prep_half(x0, qx0, 2, 4)

            # transpose: [128s, m*KE] -> [128k, m*(KT+1), 128s],
            # chunk j = s*(KT+1)+kt.  Quad 0's transposes ride the scalar
            # HWDGE ring so the sync ring is free for the weight-tile
            # transposes the first matmuls are gated on.
            qxT0 = qtpool.tile([128, QS, KT + 1, 128], BF16, name="qxT", tag="qxT")
            nc.scalar.dma_start_transpose(qxT0[:, 0:2], qx0[:, 0:2, :])
            nc.scalar.dma_start_transpose(qxT0[:, 2:4], qx0[:, 2:4, :])

            # ---- weight quantization + transpose (one-time) ----
            # Three exact ACT passes (DVE stays free for x prep):
            #   tw  = fl(w*s_w)              (Copy, scale)
            #   tw  = fl(tw + MAGIC)         (Copy, bias)   -> RNE round-to-int + M
            #   qw  = Sign(tw - MAGIC)       (Sign, bias)
            # tw - MAGIC is an exact small integer, and clip(i,-1,1) == Sign(i)
            # for integers, so this matches the reference's round-then-clip
            # with the same rounding points.
            for c in range(WC):
                w_s = w_tiles[c]
                tw = wpool.tile([128, 2, k], F32, name="tw", tag="tw")
                nc.scalar.activation(tw[:], w_s[:], ACTF.Copy, scale=sw_ap)
                nc.scalar.activation(tw[:], tw[:], ACTF.Copy, bias=MAGIC)
                qw = wpool.tile([128, 2, k], BF16, name="qw", tag="qw")
                nc.scalar.activation(qw[:], tw[:], ACTF.Sign, bias=neg_magic)
                # one batched xbar transpose per n-tile: [128n, 1024k] ->
                # [128k, KT, 128n] (out row r = j*128+p, verified on hw)
                nc.sync.dma_start_transpose(qwT[:, 2 * c], qw[:, 0, :])
                nc.sync.dma_start_transpose(qwT[:, 2 * c + 1], qw[:, 1, :])

            # ---- main pipeline over token quads ----
            def emit_bias_mm(ps_list, qxT, s):
                for h in range(NH):
                    nc.tensor.matmul(
                        ps_list[h][:],
                        qxT[0:3, s, KT, :],
                        brows[:, h * 512:(h + 1) * 512],
                        start=False,
                        stop=True,
                    )

            def emit_output(q, s, ps_list, fs_ap):
                o = opool.tile([128, n], F32, name="o", tag="o")
                for h in range(NH):
                    nc.scalar.activation(
                        o[:, h * 512:(h + 1) * 512], ps_list[h][:],
                        ACTF.Copy, scale=fs_ap,
                    )
                nc.gpsimd.dma_start(out_q[q][:, s, :], o[:])

            for q in range(NQ):
                # look-ahead x load (2 quads ahead; xpool bufs=3)
                if q + 2 < NQ:
                    xn = xpool.tile([128, QS, k], F32, name="x_s", tag="x_s")
                    x_tiles[q + 2] = xn
                    nc.scalar.dma_start(xn[:], x_q[q + 2])

                if q == 0:
                    # phase-split matmuls: all h=0 first (needs only the
                    # first 4 transposed weight tiles), then all h=1.  Each
                    # kt loop interleaves a PAIR of psum banks so no two
                    # consecutive matmuls hit the same bank.
                    ps = {}
                    for h in range(NH):
                        for s0 in (0, 2):
                            for s in (s0, s0 + 1):
                                ps[(s, h)] = psum_pool.tile(
                                    [128, 512], F32, name=f"ps{h}", tag=f"ps{h}"
                                )
                            for kt in range(KT):
                                for s in (s0, s0 + 1):
                                    nc.tensor.matmul(
                                        ps[(s, h)][:],
                                        qxT0[:, s, kt, :],
                                        qwT[:, 4 * h:4 * h + 4, kt, :],
                                        start=(kt == 0),
                                        stop=False,
                                    )
                        # this h's bias rows right after its phase: real PE
                        # work that bridges the wait for the other phase's
                        # weight transposes
                        for s in range(QS):
                            nc.tensor.matmul(
                                ps[(s, h)][:],
                                qxT0[0:3, s, KT, :],
                                brows[:, h * 512:(h + 1) * 512],
                                start=False,
                                stop=True,
                            )
                    for s in range(QS):
                        ps_list = [ps[(s, h)] for h in range(NH)]
                        fs_ap = (fs0a if s < 2 else fs0b)[:, s % 2:s % 2 + 1]
                        emit_output(q, s, ps_list, fs_ap)
                    continue

                # steady-state quad: batched prep on DVE, one transpose,
                # kt-interleaved matmuls per s-group.
                x_s = x_tiles[q]
                qx = qpool.tile([128, QS, KE], BF16, name="qx", tag="qx")
                fs = prep_half(x_s, qx, 0, QS)
                qxT = qtpool.tile([128, QS, KT + 1, 128], BF16, name="qxT", tag="qxT")
                nc.sync.dma_start_transpose(qxT[:], qx[:])

                for s in range(QS):
                    ps_list = [
                        psum_pool.tile([128, 512], F32, name=f"ps{h}", tag=f"ps{h}")
                        for h in range(NH)
                    ]
                    for kt in range(KT):
                        for h in range(NH):
                            nc.tensor.matmul(
                                ps_list[h][:],
                                qxT[:, s, kt, :],
                                qwT[:, 4 * h:4 * h + 4, kt, :],
                                start=(kt == 0),
                                stop=False,
                            )
                    emit_bias_mm(ps_list, qxT, s)
                    emit_output(q, s, ps_list, fs[:, s:s + 1])

    nc.compile()
    return nc


def host_consts(weight):
    """The one input-derived scalar: s_w, matching the reference's fp32 mean."""
    try:
        import jax
        import jax.numpy as jnp

        with jax.default_device(jax.devices("cpu")[0]):
            mean_abs = np.float32(
                jax.device_get(jnp.mean(jnp.abs(jnp.asarray(weight, dtype=jnp.float32))))
            )
    except Exception:
        mean_abs = np.float32(np.mean(np.abs(weight), dtype=np.float32))
    mean_c = np.maximum(mean_abs, np.float32(EPS))
    sw = np.float32(1.0) / mean_c          # s_w, the weight quant scale
    wdiv = np.float32(1.0) / sw            # 1/s_w (the ternary unit value)
    k1 = wdiv / np.float32(127.0)          # output scale = cc * k1
    k2 = np.float32(1.0) / k1              # bias e-row scale (= 1/k1)
    row = np.zeros((8,), np.float32)
    row[0], row[1], row[2], row[3] = sw, np.float32(127.0), k1, k2
    row[4] = np.float32(-MAGIC)
    return np.tile(row[None, :], (128, 1)).copy()


def host_brows(bias):
    """bf16 hi/lo split of the bias for the bias-via-matmul rows."""
    import ml_dtypes

    b = np.asarray(bias, dtype=np.float32)
    b_hi = b.astype(ml_dtypes.bfloat16)
    b_lo = (b - b_hi.astype(np.float32)).astype(ml_dtypes.bfloat16)
    return np.stack([b_hi, b_lo, b_hi], axis=0)


_NC_CACHE = {}


def _get_nc():
    if "nc" not in _NC_CACHE:
        _NC_CACHE["nc"] = build()
    return _NC_CACHE["nc"]


def make_in_maps(x, weight, bias):
    x = np.ascontiguousarray(x, dtype=np.float32)
    weight = np.ascontiguousarray(weight, dtype=np.float32)
    bias = np.ascontiguousarray(bias, dtype=np.float32)
    consts = host_consts(weight)
    brows = host_brows(bias)
    return [
        {"x": x[i], "w": weight, "brows": brows, "consts": consts}
        for i in range(N_CORES)
    ]


def kernel(x, weight, bias, **kwargs):
    nc = _get_nc()
    in_maps = make_in_maps(x, weight, bias)
    last_err = None
    for _attempt in range(3):
        try:
            res = run_bass_kernel_spmd(nc, in_maps, list(range(N_CORES)))
            return np.stack([res.results[i]["out"] for i in range(N_CORES)], axis=0)
        except Exception as e:  # transient NRT device errors: retry
            last_err = e
    raise last_err


# revision 28
# speedup vs baseline: 1.0295x; 1.0013x over previous
"""BitLinear (fake-quant straight-through) Trainium2 kernel.

Math (per the reference nn module):
  dqx = round(x * s_x) / s_x         s_x = 127 / clip(rowabsmax(x), 1e-5)   (per token row)
  dqw = clip(round(w * s_w), -1, 1) / s_w    s_w = 1 / clip(mean(|w|), 1e-5)  (per tensor)
  out = dqx @ dqw.T + bias

Key facts this kernel exploits:
  * round(x*s_x) is an integer in [-127, 127] and clip(round(w*s_w)) is in
    {-1, 0, 1}; both are EXACT in bf16, and the matmul accumulates in fp32
    PSUM where all partial sums (<= 2^17) are exact integers.  So the heavy
    matmul runs at bf16 PE rate with zero quantization-path error; the
    per-token / per-tensor scales are applied to the (exact) integer matmul
    result afterwards.
  * round-half-even == fp32 RNE, so `round(v)` is computed exactly as
    `(v + 1.5*2^23) - 1.5*2^23` with two fp32 ALU stages (no Round op needed).
  * the bias lands in PSUM pre-scale as three extra contraction rows
    ([e_hi, e_lo, e_hi] x [b_hi, b_hi, b_lo], e = per-token 1/out-scale split
    bf16 hi/lo) so no engine downstream of the matmul does elementwise work
    beyond the single scaled PSUM evacuation.  (A gpsimd bias-add variant
    measured SLOWER: it saturates the in-order gpsimd queue and triples HAM
    power-throttle windows.)

Sharding: data parallel over the batch dim; core i computes batch element i
with the full weight.  No collectives; the host scatters x and gathers out.

Pipeline structure: tokens are processed in "quads" (4 x 128 = 512 tokens).
Engine assignment (each pipeline stage owns an engine; upstream x-prep never
shares an engine with anything downstream of the matmul):
  scalar : x input DMA triggers (own HWDGE ring), one-time weight scale,
           PSUM evacuation with per-token output scale
  vector : absmax reduce, scales, quantize (round via magic), weight round/clip
  sync   : xbar transposes only (x quads and the one-time weight tiles)
  tensor : matmuls (bf16 exact-integer)
  gpsimd : one-time weight-chunk input DMA, per-s output stores (SWDGE)

Startup is latency-optimized: x quad 0 and the weight tensor stream in on
separate DMA rings; quad 0's x prep is split in two halves; its matmuls run
as h0-then-h1 phases (each kt-loop interleaves a PAIR of PSUM banks --
back-to-back matmuls into the SAME bank run at half rate) so they can begin
as soon as the first four weight n-tiles are transposed.  Weight chunks 0-1
quantize on DVE (fast path for phase h0), chunks 2-3 on the otherwise-idle
gpsimd so DVE can move straight on to quad-1 prep.

The PE's power controller (HAM) is a duty-cycle ratchet: it grants full
clock only after sustained matmul activity and regresses to ~50% duty after
idle periods (measured: 6.8us full / 10.2us half alternation growing to
80us+ full windows).  A stream of small dummy matmuls on zeros pre-warms the
ratchet during the load/prep phase so the real matmuls start at full rate,
with a second short filler batch bridging the phase-A -> phase-B weight wait.

The per-tensor weight scale s_w is the one input-derived scalar computed on
the host (it must match the reference's fp32 mean reduction to ~1 ulp, which
an on-device sequential reduction cannot guarantee; a 1e-6 relative error in
s_w flips ternary weights and produces visible output error).  It is passed
in through a small constants tensor, so the compiled program is input-
independent.
"""

import numpy as np

from concourse import bacc, bass, mybir, tile
from concourse.bass_utils import run_bass_kernel_spmd

F32 = mybir.dt.float32
BF16 = mybir.dt.bfloat16
ALU = mybir.AluOpType
ACTF = mybir.ActivationFunctionType

MAGIC = 12582912.0  # 1.5 * 2**23: fp32 RNE round-to-integer constant
EPS = 1e-05

B, S, K, N = 8, 4096, 1024, 1024
N_CORES = 8
QS = 4  # token tiles per quad


def build(s_tokens=S, k=K, n=N):
    """Build the single-core SPMD program: x[s_tokens,k] @ w[n,k]^T quantized."""
    nc = bacc.Bacc("TRN2", target_bir_lowering=False, debug=False)

    x_d = nc.dram_tensor("x", [s_tokens, k], F32, kind="ExternalInput").ap()
    w_d = nc.dram_tensor("w", [n, k], F32, kind="ExternalInput").ap()
    brows_d = nc.dram_tensor("brows", [3, n], BF16, kind="ExternalInput").ap()
    consts_d = nc.dram_tensor("consts", [128, 8], F32, kind="ExternalInput").ap()
    out_d = nc.dram_tensor("out", [s_tokens, n], F32, kind="ExternalOutput").ap()

    KT = k // 128          # contraction tiles
    NT = n // 128          # weight row tiles
    NH = n // 512          # psum-bank halves of the output feature dim
    NQ = s_tokens // (128 * QS)  # quads
    WC = 4                 # weight chunks (2 n-tiles each)
    KE = k + 128           # one extra transpose chunk for the bias e-rows

    x_q = x_d.rearrange("(q s p) k -> q p s k", s=QS, p=128)
    out_q = out_d.rearrange("(q s p) n -> q p s n", s=QS, p=128)
    w_c = w_d.rearrange("(c t p) k -> c p t k", t=2, p=128)

    with tile.TileContext(nc) as tc:
        with (
            tc.tile_pool(name="static", bufs=1) as static,
            tc.tile_pool(name="wpool", bufs=2) as wpool,
            tc.tile_pool(name="xpool", bufs=3) as xpool,
            tc.tile_pool(name="qpool", bufs=2) as qpool,
            tc.tile_pool(name="qtpool", bufs=2) as qtpool,
            tc.tile_pool(name="opool", bufs=6) as opool,
            tc.tile_pool(name="vpool", bufs=8) as vpool,
            tc.tile_pool(name="psum", bufs=4, space="PSUM") as psum_pool,
        ):
            consts = static.tile([128, 8], F32)
            nc.scalar.dma_start(consts[:], consts_d[:])
            # bias rows for the bias-via-matmul contraction: [b_hi, b_lo, b_hi]
            brows = static.tile([3, n], BF16)
            nc.scalar.dma_start(brows[:], brows_d[:])

            # qwT[kpart, nt, kt, n128] = quantized weight, transposed:
            # k on partitions; [nt, kt] outer so each n-tile's transpose
            # writes a contiguous block
            qwT = static.tile([128, NT, KT, 128], BF16)

            sw_ap = consts[:, 0:1]    # s_w  (weight quant scale)
            k1 = consts[:, 2:3]       # (1/s_w) / 127  (output scale factor)
            k2 = consts[:, 3:4]       # 127 * s_w  (= 1/k1: bias row scale)
            neg_magic = consts[:, 4:5]  # -MAGIC (bias operand for ACT Sign)

            # ---- input loads: x quad 0 (two halves, for startup latency)
            # and quad 1 on the scalar HWDGE ring; weight chunks on the
            # gpsimd SWDGE ring so the two streams proceed in parallel.
            x_tiles = {}
            x0 = xpool.tile([128, QS, k], F32, name="x_s", tag="x_s")
            x_tiles[0] = x0
            nc.gpsimd.dma_start(x0[:, 0:2, :], x_q[0][:, 0:2, :])
            nc.scalar.dma_start(x0[:, 2:4, :], x_q[0][:, 2:4, :])

            x1 = xpool.tile([128, QS, k], F32, name="x_s", tag="x_s")
            x_tiles[1] = x1
            nc.scalar.dma_start(x1[:], x_q[1])

            w_tiles = []
            for c in range(WC):
                w_s = wpool.tile([128, 2, k], F32, name="w_s", tag="w_s", bufs=4)
                nc.gpsimd.dma_start(w_s[:], w_c[c])
                w_tiles.append(w_s)

            # ---- x prep (DVE only); the absmax reduce is split in two so
            # the first half can start as soon as the first of the quad's
            # two load DMAs lands.
            def prep_half(x_s, qx, lo, hi, split_reduce=False):
                m = hi - lo
                c_ = vpool.tile([128, m], F32, name="c")
                if split_reduce:
                    h2 = (lo + hi) // 2
                    nc.vector.tensor_reduce(
                        c_[:, 0:h2 - lo], x_s[:, lo:h2, :], mybir.AxisListType.X,
                        ALU.max, apply_absolute_value=True,
                    )
                    nc.vector.tensor_reduce(
                        c_[:, h2 - lo:m], x_s[:, h2:hi, :], mybir.AxisListType.X,
                        ALU.max, apply_absolute_value=True,
                    )
                else:
                    nc.vector.tensor_reduce(
                        c_[:], x_s[:, lo:hi, :], mybir.AxisListType.X, ALU.max,
                        apply_absolute_value=True,
                    )
                cc = vpool.tile([128, m], F32, name="cc")
                nc.vector.tensor_scalar_max(cc[:], c_[:], EPS)
                rc = vpool.tile([128, m], F32, name="rc")
                nc.vector.reciprocal(rc[:], cc[:])
                ss = vpool.tile([128, m], F32, name="ss")
                nc.vector.tensor_scalar_mul(ss[:], rc[:], 127.0)
                fs = vpool.tile([128, m], F32, name="fs", tag="fs", bufs=4)
                nc.vector.tensor_scalar_mul(fs[:], cc[:], k1)
                # e = 1/fs = rc*k2 split into bf16 hi+lo, fed into the matmul
                # as extra contraction rows so the bias lands in PSUM pre-scale
                ef = vpool.tile([128, m], F32, name="ef")
                nc.vector.tensor_scalar_mul(ef[:], rc[:], k2)
                ehi = vpool.tile([128, m], BF16, name="ehi")
                nc.vector.tensor_copy(ehi[:], ef[:])
                elo = vpool.tile([128, m], BF16, name="elo")
                nc.vector.tensor_tensor(elo[:], ef[:], ehi[:], ALU.subtract)
                for s in range(lo, hi):
                    nc.vector.tensor_scalar(
                        x_s[:, s, :], x_s[:, s, :], ss[:, s - lo:s - lo + 1],
                        MAGIC, ALU.mult, ALU.add,
                    )
                # round(x*s_x) via magic constant, to bf16 into cols 0..k;
                # cols k..k+3 get the bias e-rows so the quad transpose also
                # produces the bias lhsT.
                nc.vector.tensor_scalar_sub(
                    qx[:, lo:hi, 0:k], x_s[:, lo:hi, :], MAGIC
                )
                nc.vector.tensor_copy(qx[:, lo:hi, k:k + 1], ehi[:])
                nc.vector.tensor_copy(qx[:, lo:hi, k + 1:k + 2], ehi[:])
                nc.vector.tensor_copy(qx[:, lo:hi, k + 2:k + 3], elo[:])
                # the rest of the extra chunk is never read by matmuls, but
                # the transpose streams it; keep it initialized
                nc.vector.memset(qx[:, lo:hi, k + 3:KE], 0.0)
                return fs

            qx0 = qpool.tile([128, QS, KE], BF16, name="qx", tag="qx")
            fs0a = prep_half(x0, qx0, 0, 2)
            fs0b = prep_half(x0, qx0, 2, 4)

            # transpose: [128s, m*KE] -> [128k, m*(KT+1), 128s],
            # chunk j = s*(KT+1)+kt
            qxT0 = qtpool.tile([128, QS, KT + 1, 128], BF16, name="qxT", tag="qxT")
            nc.sync.dma_start_transpose(qxT0[:, 0:2], qx0[:, 0:2, :])
            nc.sync.dma_start_transpose(qxT0[:, 2:4], qx0[:, 2:4, :])

            # ---- weight quantization + transpose (one-time) ----
            # Three exact ACT passes (DVE stays free for x prep):
            #   tw  = fl(w*s_w)              (Copy, scale)
            #   tw  = fl(tw + MAGIC)         (Copy, bias)   -> RNE round-to-int + M
            #   qw  = Sign(tw - MAGIC)       (Sign, bias)
            # tw - MAGIC is an exact small integer, and clip(i,-1,1) == Sign(i)
            # for integers, so this matches the reference's round-then-clip
            # with the same rounding points.
            for c in range(WC):
                w_s = w_tiles[c]
                tw = wpool.tile([128, 2, k], F32, name="tw", tag="tw")
                nc.scalar.activation(tw[:], w_s[:], ACTF.Copy, scale=sw_ap)
                nc.scalar.activation(tw[:], tw[:], ACTF.Copy, bias=MAGIC)
                qw = wpool.tile([128, 2, k], BF16, name="qw", tag="qw")
                nc.scalar.activation(qw[:], tw[:], ACTF.Sign, bias=neg_magic)
                # one batched xbar transpose per n-tile: [128n, 1024k] ->
                # [128k, KT, 128n] (out row r = j*128+p, verified on hw)
                nc.sync.dma_start_transpose(qwT[:, 2 * c], qw[:, 0, :])
                nc.sync.dma_start_transpose(qwT[:, 2 * c + 1], qw[:, 1, :])

            # ---- main pipeline over token quads ----
            def emit_bias_mm(ps_list, qxT, s):
                for h in range(NH):
                    nc.tensor.matmul(
                        ps_list[h][:],
                        qxT[0:3, s, KT, :],
                        brows[:, h * 512:(h + 1) * 512],
                        start=False,
                        stop=True,
                    )

            def emit_output(q, s, ps_list, fs_ap):
                o = opool.tile([128, n], F32, name="o", tag="o")
                for h in range(NH):
                    nc.scalar.activation(
                        o[:, h * 512:(h + 1) * 512], ps_list[h][:],
                        ACTF.Copy, scale=fs_ap,
                    )
                nc.gpsimd.dma_start(out_q[q][:, s, :], o[:])

            for q in range(NQ):
                # look-ahead x load (2 quads ahead; xpool bufs=3)
                if q + 2 < NQ:
                    xn = xpool.tile([128, QS, k], F32, name="x_s", tag="x_s")
                    x_tiles[q + 2] = xn
                    nc.scalar.dma_start(xn[:], x_q[q + 2])

                if q == 0:
                    # phase-split matmuls: all h=0 first (needs only the
                    # first 4 transposed weight tiles), then all h=1.  Each
                    # kt loop interleaves a PAIR of psum banks so no two
                    # consecutive matmuls hit the same bank.
                    ps = {}
                    for h in range(NH):
                        for s0 in (0, 2):
                            for s in (s0, s0 + 1):
                                ps[(s, h)] = psum_pool.tile(
                                    [128, 512], F32, name=f"ps{h}", tag=f"ps{h}"
                                )
                            for kt in range(KT):
                                for s in (s0, s0 + 1):
                                    nc.tensor.matmul(
                                        ps[(s, h)][:],
                                        qxT0[:, s, kt, :],
                                        qwT[:, 4 * h:4 * h + 4, kt, :],
                                        start=(kt == 0),
                                        stop=False,
                                    )
                        # this h's bias rows right after its phase: real PE
                        # work that bridges the wait for the other phase's
                        # weight transposes
                        for s in range(QS):
                            nc.tensor.matmul(
                                ps[(s, h)][:],
                                qxT0[0:3, s, KT, :],
                                brows[:, h * 512:(h + 1) * 512],
                                start=False,
                                stop=True,
                            )
                    for s in range(QS):
                        ps_list = [ps[(s, h)] for h in range(NH)]
                        fs_ap = (fs0a if s < 2 else fs0b)[:, s % 2:s % 2 + 1]
                        emit_output(q, s, ps_list, fs_ap)
                    continue

                # steady-state quad: batched prep on DVE, one transpose,
                # kt-interleaved matmuls per s-group.
                x_s = x_tiles[q]
                qx = qpool.tile([128, QS, KE], BF16, name="qx", tag="qx")
                fs = prep_half(x_s, qx, 0, QS)
                qxT = qtpool.tile([128, QS, KT + 1, 128], BF16, name="qxT", tag="qxT")
                nc.sync.dma_start_transpose(qxT[:], qx[:])

                for s in range(QS):
                    ps_list = [
                        psum_pool.tile([128, 512], F32, name=f"ps{h}", tag=f"ps{h}")
                        for h in range(NH)
                    ]
                    for kt in range(KT):
                        for h in range(NH):
                            nc.tensor.matmul(
                                ps_list[h][:],
                                qxT[:, s, kt, :],
                                qwT[:, 4 * h:4 * h + 4, kt, :],
                                start=(kt == 0),
                                stop=False,
                            )
                    emit_bias_mm(ps_list, qxT, s)
                    emit_output(q, s, ps_list, fs[:, s:s + 1])

    nc.compile()
    return nc


def host_consts(weight):
    """The one input-derived scalar: s_w, matching the reference's fp32 mean."""
    try:
        import jax
        import jax.numpy as jnp

        with jax.default_device(jax.devices("cpu")[0]):
            mean_abs = np.float32(
                jax.device_get(jnp.mean(jnp.abs(jnp.asarray(weight, dtype=jnp.float32))))
            )
    except Exception:
        mean_abs = np.float32(np.mean(np.abs(weight), dtype=np.float32))
    mean_c = np.maximum(mean_abs, np.float32(EPS))
    sw = np.float32(1.0) / mean_c          # s_w, the weight quant scale
    wdiv = np.float32(1.0) / sw            # 1/s_w (the ternary unit value)
    k1 = wdiv / np.float32(127.0)          # output scale = cc * k1
    k2 = np.float32(1.0) / k1              # bias e-row scale (= 1/k1)
    row = np.zeros((8,), np.float32)
    row[0], row[1], row[2], row[3] = sw, np.float32(127.0), k1, k2
    row[4] = np.float32(-MAGIC)
    return np.tile(row[None, :], (128, 1)).copy()


def host_brows(bias):
    """bf16 hi/lo split of the bias for the bias-via-matmul rows."""
    import ml_dtypes

    b = np.asarray(bias, dtype=np.float32)
    b_hi = b.astype(ml_dtypes.bfloat16)
    b_lo = (b - b_hi.astype(np.float32)).astype(ml_dtypes.bfloat16)
    return np.stack([b_hi, b_lo, b_hi], axis=0)


_NC_CACHE = {}


def _get_nc():
    if "nc" not in _NC_CACHE:
        _NC_CACHE["nc"] = build()
    return _NC_CACHE["nc"]


def make_in_maps(x, weight, bias):
    x = np.ascontiguousarray(x, dtype=np.float32)
    weight = np.ascontiguousarray(weight, dtype=np.float32)
    bias = np.ascontiguousarray(bias, dtype=np.float32)
    consts = host_consts(weight)
    brows = host_brows(bias)
    return [
        {"x": x[i], "w": weight, "brows": brows, "consts": consts}
        for i in range(N_CORES)
    ]


def kernel(x, weight, bias, **kwargs):
    nc = _get_nc()
    in_maps = make_in_maps(x, weight, bias)
    last_err = None
    for _attempt in range(3):
        try:
            res = run_bass_kernel_spmd(nc, in_maps, list(range(N_CORES)))
            return np.stack([res.results[i]["out"] for i in range(N_CORES)], axis=0)
        except Exception as e:  # transient NRT device errors: retry
            last_err = e
    raise last_err


# revision 29
# speedup vs baseline: 1.0499x; 1.0198x over previous
"""BitLinear (fake-quant straight-through) Trainium2 kernel.

Math (per the reference nn module):
  dqx = round(x * s_x) / s_x         s_x = 127 / clip(rowabsmax(x), 1e-5)   (per token row)
  dqw = clip(round(w * s_w), -1, 1) / s_w    s_w = 1 / clip(mean(|w|), 1e-5)  (per tensor)
  out = dqx @ dqw.T + bias

Key facts this kernel exploits:
  * round(x*s_x) is an integer in [-127, 127] and clip(round(w*s_w)) is in
    {-1, 0, 1}; both are EXACT in bf16, and the matmul accumulates in fp32
    PSUM where all partial sums (<= 2^17) are exact integers.  So the heavy
    matmul runs at bf16 PE rate with zero quantization-path error; the
    per-token / per-tensor scales are applied to the (exact) integer matmul
    result afterwards.
  * round-half-even == fp32 RNE, so `round(v)` is computed exactly as
    `(v + 1.5*2^23) - 1.5*2^23` with two fp32 ALU stages (no Round op needed).
  * the bias lands in PSUM pre-scale as three extra contraction rows
    ([e_hi, e_lo, e_hi] x [b_hi, b_hi, b_lo], e = per-token 1/out-scale split
    bf16 hi/lo) so no engine downstream of the matmul does elementwise work
    beyond the single scaled PSUM evacuation.  (A gpsimd bias-add variant
    measured SLOWER: it saturates the in-order gpsimd queue and triples HAM
    power-throttle windows.)

Sharding: data parallel over the batch dim; core i computes batch element i
with the full weight.  No collectives; the host scatters x and gathers out.

Pipeline structure: tokens are processed in "quads" (4 x 128 = 512 tokens).
Engine assignment (each pipeline stage owns an engine; upstream x-prep never
shares an engine with anything downstream of the matmul):
  scalar : x input DMA triggers (own HWDGE ring), one-time weight scale,
           PSUM evacuation with per-token output scale
  vector : absmax reduce, scales, quantize (round via magic), weight round/clip
  sync   : xbar transposes only (x quads and the one-time weight tiles)
  tensor : matmuls (bf16 exact-integer)
  gpsimd : one-time weight-chunk input DMA, per-s output stores (SWDGE)

Startup is latency-optimized: x quad 0 and the weight tensor stream in on
separate DMA rings; quad 0's x prep is split in two halves; its matmuls run
as h0-then-h1 phases (each kt-loop interleaves a PAIR of PSUM banks --
back-to-back matmuls into the SAME bank run at half rate) so they can begin
as soon as the first four weight n-tiles are transposed.  Weight chunks 0-1
quantize on DVE (fast path for phase h0), chunks 2-3 on the otherwise-idle
gpsimd so DVE can move straight on to quad-1 prep.

The PE's power controller (HAM) is a duty-cycle ratchet: it grants full
clock only after sustained matmul activity and regresses to ~50% duty after
idle periods (measured: 6.8us full / 10.2us half alternation growing to
80us+ full windows).  A stream of small dummy matmuls on zeros pre-warms the
ratchet during the load/prep phase so the real matmuls start at full rate,
with a second short filler batch bridging the phase-A -> phase-B weight wait.

The per-tensor weight scale s_w is the one input-derived scalar computed on
the host (it must match the reference's fp32 mean reduction to ~1 ulp, which
an on-device sequential reduction cannot guarantee; a 1e-6 relative error in
s_w flips ternary weights and produces visible output error).  It is passed
in through a small constants tensor, so the compiled program is input-
independent.
"""

import numpy as np

from concourse import bacc, bass, mybir, tile
from concourse.bass_utils import run_bass_kernel_spmd

F32 = mybir.dt.float32
BF16 = mybir.dt.bfloat16
ALU = mybir.AluOpType
ACTF = mybir.ActivationFunctionType

MAGIC = 12582912.0  # 1.5 * 2**23: fp32 RNE round-to-integer constant
EPS = 1e-05

B, S, K, N = 8, 4096, 1024, 1024
N_CORES = 8
QS = 4  # token tiles per quad


def build(s_tokens=S, k=K, n=N):
    """Build the single-core SPMD program: x[s_tokens,k] @ w[n,k]^T quantized."""
    nc = bacc.Bacc("TRN2", target_bir_lowering=False, debug=False)

    x_d = nc.dram_tensor("x", [s_tokens, k], F32, kind="ExternalInput").ap()
    w_d = nc.dram_tensor("w", [n, k], F32, kind="ExternalInput").ap()
    brows_d = nc.dram_tensor("brows", [3, n], BF16, kind="ExternalInput").ap()
    consts_d = nc.dram_tensor("consts", [128, 8], F32, kind="ExternalInput").ap()
    out_d = nc.dram_tensor("out", [s_tokens, n], F32, kind="ExternalOutput").ap()

    KT = k // 128          # contraction tiles
    NT = n // 128          # weight row tiles
    NH = n // 512          # psum-bank halves of the output feature dim
    NQ = s_tokens // (128 * QS)  # quads
    WC = 4                 # weight chunks (2 n-tiles each)
    KE = k + 128           # one extra transpose chunk for the bias e-rows

    x_q = x_d.rearrange("(q s p) k -> q p s k", s=QS, p=128)
    out_q = out_d.rearrange("(q s p) n -> q p s n", s=QS, p=128)
    w_c = w_d.rearrange("(c t p) k -> c p t k", t=2, p=128)

    with tile.TileContext(nc) as tc:
        with (
            tc.tile_pool(name="static", bufs=1) as static,
            tc.tile_pool(name="wpool", bufs=2) as wpool,
            tc.tile_pool(name="xpool", bufs=3) as xpool,
            tc.tile_pool(name="qpool", bufs=2) as qpool,
            tc.tile_pool(name="qtpool", bufs=2) as qtpool,
            tc.tile_pool(name="opool", bufs=6) as opool,
            tc.tile_pool(name="vpool", bufs=8) as vpool,
            tc.tile_pool(name="psum", bufs=4, space="PSUM") as psum_pool,
        ):
            consts = static.tile([128, 8], F32)
            nc.scalar.dma_start(consts[:], consts_d[:])
            # bias rows for the bias-via-matmul contraction: [b_hi, b_lo, b_hi]
            brows = static.tile([3, n], BF16)
            nc.scalar.dma_start(brows[:], brows_d[:])

            # qwT[kpart, nt, kt, n128] = quantized weight, transposed:
            # k on partitions; [nt, kt] outer so each n-tile's transpose
            # writes a contiguous block
            qwT = static.tile([128, NT, KT, 128], BF16)

            sw_ap = consts[:, 0:1]    # s_w  (weight quant scale)
            k1 = consts[:, 2:3]       # (1/s_w) / 127  (output scale factor)
            k2 = consts[:, 3:4]       # 127 * s_w  (= 1/k1: bias row scale)
            neg_magic = consts[:, 4:5]  # -MAGIC (bias operand for ACT Sign)

            # ---- input loads: x quad 0 (two halves, for startup latency)
            # and quad 1 on the scalar HWDGE ring; weight chunks on the
            # gpsimd SWDGE ring so the two streams proceed in parallel.
            x_tiles = {}
            x0 = xpool.tile([128, QS, k], F32, name="x_s", tag="x_s")
            x_tiles[0] = x0
            nc.gpsimd.dma_start(x0[:, 0:2, :], x_q[0][:, 0:2, :])
            nc.scalar.dma_start(x0[:, 2:4, :], x_q[0][:, 2:4, :])

            x1 = xpool.tile([128, QS, k], F32, name="x_s", tag="x_s")
            x_tiles[1] = x1
            nc.scalar.dma_start(x1[:], x_q[1])

            w_tiles = []
            for c in range(WC):
                w_s = wpool.tile([128, 2, k], F32, name="w_s", tag="w_s", bufs=4)
                nc.gpsimd.dma_start(w_s[:], w_c[c])
                w_tiles.append(w_s)

            # ---- x prep (DVE only); the absmax reduce is split in two so
            # the first half can start as soon as the first of the quad's
            # two load DMAs lands.
            def prep_half(x_s, qx, lo, hi, split_reduce=False):
                m = hi - lo
                c_ = vpool.tile([128, m], F32, name="c")
                if split_reduce:
                    h2 = (lo + hi) // 2
                    nc.vector.tensor_reduce(
                        c_[:, 0:h2 - lo], x_s[:, lo:h2, :], mybir.AxisListType.X,
                        ALU.max, apply_absolute_value=True,
                    )
                    nc.vector.tensor_reduce(
                        c_[:, h2 - lo:m], x_s[:, h2:hi, :], mybir.AxisListType.X,
                        ALU.max, apply_absolute_value=True,
                    )
                else:
                    nc.vector.tensor_reduce(
                        c_[:], x_s[:, lo:hi, :], mybir.AxisListType.X, ALU.max,
                        apply_absolute_value=True,
                    )
                cc = vpool.tile([128, m], F32, name="cc")
                nc.vector.tensor_scalar_max(cc[:], c_[:], EPS)
                rc = vpool.tile([128, m], F32, name="rc")
                nc.vector.reciprocal(rc[:], cc[:])
                ss = vpool.tile([128, m], F32, name="ss")
                nc.vector.tensor_scalar_mul(ss[:], rc[:], 127.0)
                fs = vpool.tile([128, m], F32, name="fs", tag="fs", bufs=4)
                nc.vector.tensor_scalar_mul(fs[:], cc[:], k1)
                # e = 1/fs = rc*k2 split into bf16 hi+lo, fed into the matmul
                # as extra contraction rows so the bias lands in PSUM pre-scale
                ef = vpool.tile([128, m], F32, name="ef")
                nc.vector.tensor_scalar_mul(ef[:], rc[:], k2)
                ehi = vpool.tile([128, m], BF16, name="ehi")
                nc.vector.tensor_copy(ehi[:], ef[:])
                elo = vpool.tile([128, m], BF16, name="elo")
                nc.vector.tensor_tensor(elo[:], ef[:], ehi[:], ALU.subtract)
                for s in range(lo, hi):
                    nc.vector.tensor_scalar(
                        x_s[:, s, :], x_s[:, s, :], ss[:, s - lo:s - lo + 1],
                        MAGIC, ALU.mult, ALU.add,
                    )
                # round(x*s_x) via magic constant, to bf16 into cols 0..k;
                # cols k..k+3 get the bias e-rows so the quad transpose also
                # produces the bias lhsT.
                nc.vector.tensor_scalar_sub(
                    qx[:, lo:hi, 0:k], x_s[:, lo:hi, :], MAGIC
                )
                nc.vector.tensor_copy(qx[:, lo:hi, k:k + 1], ehi[:])
                nc.vector.tensor_copy(qx[:, lo:hi, k + 1:k + 2], ehi[:])
                nc.vector.tensor_copy(qx[:, lo:hi, k + 2:k + 3], elo[:])
                # the rest of the extra chunk is never read by matmuls, but
                # the transpose streams it; keep it initialized
                nc.vector.memset(qx[:, lo:hi, k + 3:KE], 0.0)
                return fs

            qx0 = qpool.tile([128, QS, KE], BF16, name="qx", tag="qx")
            fs0a = prep_half(x0, qx0, 0, 2)
            fs0b = prep_half(x0, qx0, 2, 4)

            # transpose: [128s, m*KE] -> [128k, m*(KT+1), 128s],
            # chunk j = s*(KT+1)+kt
            qxT0 = qtpool.tile([128, QS, KT + 1, 128], BF16, name="qxT", tag="qxT")
            nc.sync.dma_start_transpose(qxT0[:, 0:2], qx0[:, 0:2, :])
            nc.sync.dma_start_transpose(qxT0[:, 2:4], qx0[:, 2:4, :])

            # ---- weight quantization + transpose (one-time) ----
            # Three exact ACT passes (DVE stays free for x prep):
            #   tw  = fl(w*s_w)              (Copy, scale)
            #   tw  = fl(tw + MAGIC)         (Copy, bias)   -> RNE round-to-int + M
            #   qw  = Sign(tw - MAGIC)       (Sign, bias)
            # tw - MAGIC is an exact small integer, and clip(i,-1,1) == Sign(i)
            # for integers, so this matches the reference's round-then-clip
            # with the same rounding points.
            for c in range(WC):
                w_s = w_tiles[c]
                tw = wpool.tile([128, 2, k], F32, name="tw", tag="tw")
                nc.scalar.activation(tw[:], w_s[:], ACTF.Copy, scale=sw_ap)
                nc.scalar.activation(tw[:], tw[:], ACTF.Copy, bias=MAGIC)
                qw = wpool.tile([128, 2, k], BF16, name="qw", tag="qw")
                nc.scalar.activation(qw[:], tw[:], ACTF.Sign, bias=neg_magic)
                # one batched xbar transpose per n-tile: [128n, 1024k] ->
                # [128k, KT, 128n] (out row r = j*128+p, verified on hw)
                nc.sync.dma_start_transpose(qwT[:, 2 * c], qw[:, 0, :])
                nc.sync.dma_start_transpose(qwT[:, 2 * c + 1], qw[:, 1, :])

            # ---- main pipeline over token quads ----
            def emit_bias_mm(ps_list, qxT, s):
                for h in range(NH):
                    nc.tensor.matmul(
                        ps_list[h][:],
                        qxT[0:3, s, KT, :],
                        brows[:, h * 512:(h + 1) * 512],
                        start=False,
                        stop=True,
                    )

            def emit_output(q, s, ps_list, fs_ap):
                o = opool.tile([128, n], F32, name="o", tag="o")
                for h in range(NH):
                    nc.scalar.activation(
                        o[:, h * 512:(h + 1) * 512], ps_list[h][:],
                        ACTF.Copy, scale=fs_ap,
                    )
                nc.gpsimd.dma_start(out_q[q][:, s, :], o[:])

            for q in range(NQ):
                # look-ahead x load (2 quads ahead; xpool bufs=3)
                if q + 2 < NQ:
                    xn = xpool.tile([128, QS, k], F32, name="x_s", tag="x_s")
                    x_tiles[q + 2] = xn
                    nc.scalar.dma_start(xn[:], x_q[q + 2])

                if q == 0:
                    # phase-split matmuls: all h=0 first (needs only the
                    # first 4 transposed weight tiles), then all h=1.  Each
                    # kt loop interleaves a PAIR of psum banks so no two
                    # consecutive matmuls hit the same bank.
                    ps = {}
                    for h in range(NH):
                        for s0 in (0, 2):
                            for s in (s0, s0 + 1):
                                ps[(s, h)] = psum_pool.tile(
                                    [128, 512], F32, name=f"ps{h}", tag=f"ps{h}"
                                )
                            for kt in range(KT):
                                for s in (s0, s0 + 1):
                                    nc.tensor.matmul(
                                        ps[(s, h)][:],
                                        qxT0[:, s, kt, :],
                                        qwT[:, 4 * h:4 * h + 4, kt, :],
                                        start=(kt == 0),
                                        stop=False,
                                    )
                        # this h's bias rows right after its phase: real PE
                        # work that bridges the wait for the other phase's
                        # weight transposes
                        for s in range(QS):
                            nc.tensor.matmul(
                                ps[(s, h)][:],
                                qxT0[0:3, s, KT, :],
                                brows[:, h * 512:(h + 1) * 512],
                                start=False,
                                stop=True,
                            )
                    for s in range(QS):
                        ps_list = [ps[(s, h)] for h in range(NH)]
                        fs_ap = (fs0a if s < 2 else fs0b)[:, s % 2:s % 2 + 1]
                        emit_output(q, s, ps_list, fs_ap)
                    continue

                # steady-state quad: batched prep on DVE, one transpose,
                # kt-interleaved matmuls per s-group.
                x_s = x_tiles[q]
                qx = qpool.tile([128, QS, KE], BF16, name="qx", tag="qx")
                fs = prep_half(x_s, qx, 0, QS)
                qxT = qtpool.tile([128, QS, KT + 1, 128], BF16, name="qxT", tag="qxT")
                nc.sync.dma_start_transpose(qxT[:], qx[:])

                for s in range(QS):
                    ps_list = [
                        psum_pool.tile([128, 512], F32, name=f"ps{h}", tag=f"ps{h}")
                        for h in range(NH)
                    ]
                    for kt in range(KT):
                        for h in range(NH):
                            nc.tensor.matmul(
                                ps_list[h][:],
                                qxT[:, s, kt, :],
                                qwT[:, 4 * h:4 * h + 4, kt, :],
                                start=(kt == 0),
                                stop=False,
                            )
                    emit_bias_mm(ps_list, qxT, s)
                    emit_output(q, s, ps_list, fs[:, s:s + 1])

    nc.compile()
    return nc


def host_consts(weight):
    """The one input-derived scalar: s_w, matching the reference's fp32 mean."""
    try:
        import jax
        import jax.numpy as jnp

        with jax.default_device(jax.devices("cpu")[0]):
            mean_abs = np.float32(
                jax.device_get(jnp.mean(jnp.abs(jnp.asarray(weight, dtype=jnp.float32))))
            )
    except Exception:
        mean_abs = np.float32(np.mean(np.abs(weight), dtype=np.float32))
    mean_c = np.maximum(mean_abs, np.float32(EPS))
    sw = np.float32(1.0) / mean_c          # s_w, the weight quant scale
    wdiv = np.float32(1.0) / sw            # 1/s_w (the ternary unit value)
    k1 = wdiv / np.float32(127.0)          # output scale = cc * k1
    k2 = np.float32(1.0) / k1              # bias e-row scale (= 1/k1)
    row = np.zeros((8,), np.float32)
    row[0], row[1], row[2], row[3] = sw, np.float32(127.0), k1, k2
    row[4] = np.float32(-MAGIC)
    return np.tile(row[None, :], (128, 1)).copy()


def host_brows(bias):
    """bf16 hi/lo split of the bias for the bias-via-matmul rows."""
    import ml_dtypes

    b = np.asarray(bias, dtype=np.float32)
    b_hi = b.astype(ml_dtypes.bfloat16)
    b_lo = (b - b_hi.astype(np.float32)).astype(ml_dtypes.bfloat16)
    return np.stack([b_hi, b_lo, b_hi], axis=0)


_NC_CACHE = {}


def _get_nc():
    if "nc" not in _NC_CACHE:
        _NC_CACHE["nc"] = build()
    return _NC_CACHE["nc"]


def make_in_maps(x, weight, bias):
    x = np.ascontiguousarray(x, dtype=np.float32)
    weight = np.ascontiguousarray(weight, dtype=np.float32)
    bias = np.ascontiguousarray(bias, dtype=np.float32)
    consts = host_consts(weight)
    brows = host_brows(bias)
    return [
        {"x": x[i], "w": weight, "brows": brows, "consts": consts}
        for i in range(N_CORES)
    ]


def kernel(x, weight, bias, **kwargs):
    import time

    nc = _get_nc()
    in_maps = make_in_maps(x, weight, bias)
    last_err = None
    for attempt in range(4):
        try:
            res = run_bass_kernel_spmd(nc, in_maps, list(range(N_CORES)))
            return np.stack([res.results[i]["out"] for i in range(N_CORES)], axis=0)
        except Exception as e:  # transient NRT device errors: back off, retry
            last_err = e
            time.sleep(2.0 * (attempt + 1))
    raise last_err
